# revision 5
# baseline (speedup 1.0000x reference)
"""CausalShapedAttention Trainium2 kernel.

y = beta * softmax(causal(q k^T / sqrt(hd))) @ v + alpha * v - gamma * MC @ v

where q,k = x @ W_attn^T (packed), v = x (reshaped to heads), MC = causal
uniform attention (row i: 1/(i+1) for j<=i).

Sharding: 16 heads / 8 cores = 2 heads per core, both batches per core.
Each core computes y columns [128c, 128c+128) of the [2, 2048, 1024] output.

Key identities used:
  softmax(s)_ij = exp(s_ij)/sum_j exp(s_ij)  (no max-sub needed: |s| < ~3)
  (MC @ v)_i = (sum_{j<=i} v_j) / (i+1)
  An extra ones-column appended to v makes the U matmul also produce the
  softmax denominator (col 64), and the Lv matmul produce i+1 (col 64).

All matmuls run with bf16 operands (fp32 matmul is 4 cycles/row on trn2 PE,
bf16 is 1) accumulating in fp32 PSUM; the dominant alpha*v output term is
added in fp32 from untouched input data.
"""

import os
import sys
import types

sys.path.insert(0, "/opt/trn_rl_repo")

import numpy as np
import ml_dtypes

B, T, C, H, HD = 2, 2048, 1024, 16, 64
NCORES = 8
HPC = H // NCORES            # heads per core = 2
TB = T // 128                # 16 row/col blocks
NW = T // 256                # 8 wide column blocks

_PROGRAM = None
LAST_EXEC_NS = None
LAST_TRACE_DIR = None


def _install_patches():
    """Work around environment quirks:
    - walrus here rejects >2 sem waits on CTRL instructions: split the Tile
      exit drain's waits across single-wait SP nops.
    - antenv.axon_hooks is absent in this image: stub it and register the
      NTFF profile hook from trn_agent_boot so trace=True works.
    """
    import concourse.tile as tile
    from concourse.vector_clock import ScopedClock, VectorClock

    if not getattr(tile.TileContext, "_dab_patched", False):
        def _patched_dab(self, tick_clock, wait_clock):
            nc = self.nc
            gc = tick_clock.global_clock
            n = len(gc)
            for i in range(n):
                v = gc[i]
                if v > 0:
                    vec = [0] * n
                    vec[i] = v
                    nop = nc.sync.nop()
                    wait_clock.add_sem_waits(
                        nop.ins, ScopedClock({None: VectorClock(vec)})
                    )
            nc.sync.drain()
            nc.all_engine_barrier()
            assert self.sems is not None
            popped = nc._tile_sem_poison_stack.pop()
            assert popped is self._sem_poison
            nc.clear_and_free_semaphores(list(self.sems.allocated().values()))
            nc.all_engine_barrier()

        tile.TileContext._drain_and_barrier = _patched_dab
        tile.TileContext._dab_patched = True

    try:
        import antenv  # noqa: F401
        if "antenv.axon_hooks" not in sys.modules:
            hooks_mod = types.ModuleType("antenv.axon_hooks")
            _h = [None]
            hooks_mod.set_axon_ntff_profile_hook = lambda h: _h.__setitem__(0, h)
            hooks_mod.get_axon_ntff_profile_hook = lambda: _h[0]
            sys.modules["antenv.axon_hooks"] = hooks_mod
            antenv.axon_hooks = hooks_mod
            from trn_agent_boot.trn_boot import _ntff_profile_via_ctypes
            hooks_mod.set_axon_ntff_profile_hook(
                _ntff_profile_via_ctypes("/opt/axon/libaxon_pjrt.so")
            )
        import concourse.bass_utils as bu
        bu.upload_artifacts = lambda d: d  # no artifact bucket here
    except Exception:
        pass


def _split_excess_waits(nc, limit=1):
    """walrus here rejects instructions with more than ~2 sem waits; split
    excess waits onto same-engine NoOps inserted just before the instruction
    (engine streams are per-engine program order, so semantics are identical).
    """
    import concourse.mybir as mybir

    n = 0
    for bb in nc.main_func.blocks:
        out = []
        for inst in bb.instructions:
            si = inst.sync_info
            if (
                si is not None
                and si.on_wait
                and len(si.on_wait) > limit
                and inst.engine != mybir.EngineType.Unassigned
            ):
                waits = list(si.on_wait)
                for w in waits[:-limit]:
                    n += 1
                    out.append(mybir.InstNoOp(
                        name=f"{inst.name}-wsplit{n}",
                        engine=inst.engine,
                        ins=[], outs=[],
                        sync_info=mybir.SyncInfo(on_wait=[w], on_update=[]),
                    ))
                inst.sync_info = mybir.SyncInfo(
                    on_wait=waits[-limit:], on_update=list(si.on_update)
                )
            out.append(inst)
        bb.instructions = out


def _build_program():
    import concourse.bass as bass
    import concourse.mybir as mybir
    import concourse.tile as tile
    from concourse.bass import ts, ds

    f32 = mybir.dt.float32
    bf16 = mybir.dt.bfloat16

    nc = bass.Bass()
    xT = nc.dram_tensor("xT", [B, 8, 128, T], bf16, kind="ExternalInput")
    w = nc.dram_tensor("w", [2, 8, 128, 128], bf16, kind="ExternalInput")
    v16 = nc.dram_tensor("v16", [B, HPC, TB, 128, 65], bf16, kind="ExternalInput")
    v32 = nc.dram_tensor("v32", [B, HPC, TB, 128, 64], f32, kind="ExternalInput")
    tri_d = nc.dram_tensor("tri", [128, 128], bf16, kind="ExternalInput")
    ones_d = nc.dram_tensor("ones", [128, 128], bf16, kind="ExternalInput")
    bg_d = nc.dram_tensor("bg", [128, 2], f32, kind="ExternalInput")
    y = nc.dram_tensor("y", [B, T, HPC * 64], f32, kind="ExternalOutput")

    with tile.TileContext(nc) as tc:
        with (
            tc.tile_pool(name="consts", bufs=1) as consts,
            tc.tile_pool(name="wpool", bufs=1) as wpool,
            tc.tile_pool(name="xtp", bufs=9) as xtp,
            tc.tile_pool(name="qk", bufs=1) as qkp,
            tc.tile_pool(name="vp", bufs=1) as vp,
            tc.tile_pool(name="pt", bufs=90) as ptp,
            tc.tile_pool(name="small", bufs=12) as small,
            tc.tile_pool(name="tmp", bufs=8) as tmp,
            tc.tile_pool(name="proj_ps", bufs=2, space="PSUM") as proj_ps,
            tc.tile_pool(name="sc_ps", bufs=2, space="PSUM") as sc_ps,
            tc.tile_pool(name="uv_ps", bufs=2, space="PSUM") as uv_ps,
        ):
            tri_t = consts.tile([128, 128], bf16, tag="tri")
            nc.sync.dma_start(tri_t[:], tri_d[:])
            ones_t = consts.tile([128, 128], bf16, tag="ones")
            nc.sync.dma_start(ones_t[:], ones_d[:])
            bg_t = consts.tile([128, 2], f32, tag="bg")
            nc.sync.dma_start(bg_t[:], bg_d[:])

            w_t = {}
            for m in range(2):
                for c in range(8):
                    w_t[m, c] = wpool.tile([128, 128], bf16, name=f"w{m}{c}", tag=f"w{m}{c}")
                    nc.sync.dma_start(w_t[m, c][:], w[m, c])

            v16_t = {}
            v32_t = {}
            for b in range(B):
                for hs in range(HPC):
                    for jb in range(TB):
                        v16_t[b, hs, jb] = vp.tile([128, 65], bf16,
                                                   name=f"v16_{b}_{hs}_{jb}", tag=f"v16_{b}_{hs}_{jb}")
                        nc.sync.dma_start(v16_t[b, hs, jb][:], v16[b, hs, jb])
                        v32_t[b, hs, jb] = vp.tile([128, 64], f32,
                                                   name=f"v32_{b}_{hs}_{jb}", tag=f"v32_{b}_{hs}_{jb}")
                        nc.sync.dma_start(v32_t[b, hs, jb][:], v32[b, hs, jb])

            qk_t = {}  # (b, m): m=0 -> Q2 [128, T], m=1 -> K2
            for b in range(B):
                for m in range(2):
                    qk_t[b, m] = qkp.tile([128, T], bf16, name=f"qk{b}{m}", tag=f"qk{b}{m}")

            for b in range(B):
                # stream x^T c-chunks for this batch
                xc = []
                for c in range(8):
                    t = xtp.tile([128, T], bf16)
                    nc.sync.dma_start(t[:], xT[b, c])
                    xc.append(t)
                # projection: qk[m][p, t] = sum_c w[m][c, p] * xT[c, t]
                for m in range(2):
                    for n in range(4):
                        ps = proj_ps.tile([128, 512], f32)
                        for c in range(8):
                            nc.tensor.matmul(
                                ps[:], w_t[m, c][:], xc[c][:, ts(n, 512)],
                                start=(c == 0), stop=(c == 7),
                            )
                        nc.scalar.copy(qk_t[b, m][:, ts(n, 512)], ps[:])

            for b in range(B):
                for hs in range(HPC):
                    p0 = 64 * hs
                    q2 = qk_t[b, 0]
                    k2 = qk_t[b, 1]
                    # scoresT blocks [j, i] + exp -> PT tiles (bf16)
                    pt_t = {}
                    for jb in range(TB):
                        iw0 = jb // 2
                        for iw in range(iw0, NW):
                            sp = sc_ps.tile([128, 256], f32)
                            nc.tensor.matmul(
                                sp[:],
                                k2[ds(p0, 64), ts(jb, 128)],
                                q2[ds(p0, 64), ts(iw, 256)],
                                start=True, stop=True,
                            )
                            ptt = ptp.tile([128, 256], bf16)
                            nc.scalar.activation(
                                ptt[:], sp[:], mybir.ActivationFunctionType.Exp
                            )
                            if iw == iw0:
                                dcol = (jb % 2) * 128
                                nc.vector.tensor_mul(
                                    ptt[:, ds(dcol, 128)],
                                    ptt[:, ds(dcol, 128)],
                                    tri_t[:],
                                )
                            pt_t[jb, iw] = ptt

                    for ib in range(TB):
                        up = uv_ps.tile([128, 65], f32, tag="ups")
                        for jb in range(ib + 1):
                            ptt = pt_t[jb, ib // 2]
                            col = (ib % 2) * 128
                            nc.tensor.matmul(
                                up[:], ptt[:, ds(col, 128)], v16_t[b, hs, jb][:],
                                start=(jb == 0), stop=(jb == ib),
                            )
                        lp = uv_ps.tile([128, 65], f32, tag="lps")
                        for jb in range(ib + 1):
                            lhs = tri_t if jb == ib else ones_t
                            nc.tensor.matmul(
                                lp[:], lhs[:], v16_t[b, hs, jb][:],
                                start=(jb == 0), stop=(jb == ib),
                            )
                        r1 = small.tile([128, 1], f32, tag="r1")
                        nc.vector.reciprocal(r1[:], up[:, ds(64, 1)])
                        r1b = small.tile([128, 1], f32, tag="r1b")
                        nc.vector.tensor_scalar_mul(r1b[:], r1[:], bg_t[:, ds(0, 1)])
                        r2 = small.tile([128, 1], f32, tag="r2")
                        nc.vector.reciprocal(r2[:], lp[:, ds(64, 1)])
                        r2g = small.tile([128, 1], f32, tag="r2g")
                        nc.vector.tensor_scalar_mul(r2g[:], r2[:], bg_t[:, ds(1, 1)])
                        t1 = tmp.tile([128, 64], f32, tag="t1")
                        nc.vector.tensor_scalar_mul(t1[:], up[:, ds(0, 64)], r1b[:])
                        t2 = tmp.tile([128, 64], f32, tag="t2")
                        nc.vector.tensor_scalar_mul(t2[:], lp[:, ds(0, 64)], r2g[:])
                        t3 = tmp.tile([128, 64], f32, tag="t3")
                        nc.vector.tensor_sub(t3[:], t1[:], t2[:])
                        yt = tmp.tile([128, 64], f32, tag="yt")
                        nc.vector.tensor_add(yt[:], t3[:], v32_t[b, hs, ib][:])
                        nc.sync.dma_start(
                            y[b, ts(ib, 128), ds(p0, 64)], yt[:]
                        )

    _split_excess_waits(nc)
    nc.finalize()
    return nc


def _prep_inputs(x, W_attn, alpha, beta, gamma):
    """Host-side sharding/layout prep. Returns per-core input maps."""
    bf = ml_dtypes.bfloat16
    x = np.asarray(x, dtype=np.float32)
    W_attn = np.asarray(W_attn, dtype=np.float32)
    alpha = float(alpha)
    beta = float(beta)
    gamma = float(gamma)

    # x^T per batch, c-chunked: [B, 8, 128, T] (shared by all cores)
    xT = np.ascontiguousarray(x.transpose(0, 2, 1).reshape(B, 8, 128, T)).astype(bf)

    tri = np.triu(np.ones((128, 128), dtype=np.float32)).astype(bf)  # j<=i
    ones = np.ones((128, 128), dtype=bf)
    bg = np.empty((128, 2), dtype=np.float32)
    bg[:, 0] = beta
    bg[:, 1] = gamma

    scale = HD ** -0.5
    in_maps = []
    for core in range(NCORES):
        h0 = HPC * core
        # w[0] = q columns for (h0, h0+1), pre-scaled; w[1] = k columns
        wq = W_attn[h0 * 64:(h0 + HPC) * 64, :].T * scale      # [C, 128]
        wk = W_attn[C + h0 * 64:C + (h0 + HPC) * 64, :].T      # [C, 128]
        wpack = np.stack([wq.reshape(8, 128, 128), wk.reshape(8, 128, 128)])
        wpack = np.ascontiguousarray(wpack).astype(bf)

        v = np.empty((B, HPC, TB, 128, 65), dtype=np.float32)
        v32 = np.empty((B, HPC, TB, 128, 64), dtype=np.float32)
        for b in range(B):
            for hs in range(HPC):
                h = h0 + hs
                vb = x[b][:, h * 64:(h + 1) * 64].reshape(TB, 128, 64)
                v[b, hs, :, :, :64] = vb
                v[b, hs, :, :, 64] = 1.0
                v32[b, hs] = alpha * vb
        in_maps.append({
            "xT": xT,
            "w": wpack,
            "v16": v.astype(bf),
            "v32": np.ascontiguousarray(v32),
            "tri": tri,
            "ones": ones,
            "bg": bg,
        })
    return in_maps


def kernel(x, W_attn, alpha, beta, gamma):
    global _PROGRAM, LAST_EXEC_NS, LAST_TRACE_DIR
    _install_patches()
    from concourse.bass_utils import run_bass_kernel_spmd

    if _PROGRAM is None:
        _PROGRAM = _build_program()
    nc = _PROGRAM

    in_maps = _prep_inputs(x, W_attn, alpha, beta, gamma)

    trace = os.environ.get("KERNEL_TRACE", "0") == "1"
    kwargs = {}
    if trace:
        trace_dir = os.environ.get("KERNEL_TRACE_DIR") or None
        if trace_dir:
            os.makedirs(trace_dir, exist_ok=True)
            kwargs["tmpdir"] = trace_dir
    res = run_bass_kernel_spmd(
        nc, in_maps, core_ids=list(range(NCORES)), trace=trace, **kwargs
    )
    LAST_EXEC_NS = res.exec_time_ns
    if trace and "tmpdir" in kwargs:
        LAST_TRACE_DIR = kwargs["tmpdir"]

    out = np.concatenate(
        [res.results[c]["y"] for c in range(NCORES)], axis=2
    )
    return np.ascontiguousarray(out, dtype=np.float32)


# revision 9
# speedup vs baseline: 1.5503x; 1.5503x over previous
"""CausalShapedAttention Trainium2 kernel.

y = beta * softmax(causal(q k^T / sqrt(hd))) @ v + alpha * v - gamma * MC @ v

where q,k = x @ W_attn^T (packed), v = x (reshaped to heads), MC = causal
uniform attention (row i: 1/(i+1) for j<=i).

Sharding: 16 heads / 8 cores = 2 heads per core, both batches per core.
Each core computes y columns [128c, 128c+128) of the [2, 2048, 1024] output.

Key identities used:
  softmax(s)_ij = exp(s_ij)/sum_j exp(s_ij)  (no max-sub needed: |s| < ~3)
  (MC @ v)_i = (sum_{j<=i} v_j) / (i+1)
  An extra ones-column appended to v makes the U matmul also produce the
  softmax denominator (col 64), and the Lv matmul produce i+1 (col 64).
  Lv (running causal sum of v) per 128-row block = tri @ v_block + prefix,
  where prefix is row 127 of the previous block's Lv (rank-1 matmul add).

All matmuls run with bf16 operands (fp32 matmul is 4 cycles/row on trn2 PE,
bf16 is 1) accumulating in fp32 PSUM; the dominant alpha*v output term is
added in fp32 from untouched input data. DRAM layouts are packed so every
DMA moves large contiguous lines (>=2KB per partition row).
"""

import os
import sys
import types

sys.path.insert(0, "/opt/trn_rl_repo")

import numpy as np
import ml_dtypes

B, T, C, H, HD = 2, 2048, 1024, 16, 64
NCORES = 8
HPC = H // NCORES            # heads per core = 2
TB = T // 128                # 16 row/col blocks
NW4 = T // 512               # 4 wide (512) column blocks

_PROGRAM = None
LAST_EXEC_NS = None
LAST_TRACE_DIR = None


def _install_patches():
    """Work around environment quirks:
    - walrus here rejects instructions with >1-2 sem waits (see
      _split_excess_waits).
    - antenv.axon_hooks is absent in this image: stub it and register the
      NTFF profile hook from trn_agent_boot so trace=True works.
    """
    try:
        import antenv  # noqa: F401
        if "antenv.axon_hooks" not in sys.modules:
            hooks_mod = types.ModuleType("antenv.axon_hooks")
            _h = [None]
            hooks_mod.set_axon_ntff_profile_hook = lambda h: _h.__setitem__(0, h)
            hooks_mod.get_axon_ntff_profile_hook = lambda: _h[0]
            sys.modules["antenv.axon_hooks"] = hooks_mod
            antenv.axon_hooks = hooks_mod
            from trn_agent_boot.trn_boot import _ntff_profile_via_ctypes
            hooks_mod.set_axon_ntff_profile_hook(
                _ntff_profile_via_ctypes("/opt/axon/libaxon_pjrt.so")
            )
        import concourse.bass_utils as bu
        bu.upload_artifacts = lambda d: d  # no artifact bucket here
    except Exception:
        pass


def _split_excess_waits(nc, limit=1):
    """walrus here rejects instructions with more than ~2 sem waits; split
    excess waits onto same-engine NoOps inserted just before the instruction
    (engine streams are per-engine program order, so semantics are identical).
    """
    import concourse.mybir as mybir

    n = 0
    for bb in nc.main_func.blocks:
        out = []
        for inst in bb.instructions:
            si = inst.sync_info
            if (
                si is not None
                and si.on_wait
                and len(si.on_wait) > limit
                and inst.engine != mybir.EngineType.Unassigned
            ):
                waits = list(si.on_wait)
                for w in waits[:-limit]:
                    n += 1
                    nop = mybir.InstNoOp(
                        name=f"{inst.name}-wsplit{n}",
                        engine=inst.engine,
                        ins=[], outs=[],
                        sync_info=mybir.SyncInfo(on_wait=[w], on_update=[]),
                    )
                    nc.register_instruction(nop)
                    out.append(nop)
                inst.sync_info = mybir.SyncInfo(
                    on_wait=waits[-limit:], on_update=list(si.on_update)
                )
            out.append(inst)
        bb.instructions = out


def _build_program():
    import concourse.bass as bass
    import concourse.mybir as mybir
    import concourse.tile as tile
    from concourse.bass import ts, ds

    f32 = mybir.dt.float32
    bf16 = mybir.dt.bfloat16
    Exp = mybir.ActivationFunctionType.Exp
    mult = mybir.AluOpType.mult

    nc = bass.Bass()
    # packed DRAM layouts: every row (partition line) is large + contiguous
    xT = nc.dram_tensor("xT", [B, 8, 128, T], bf16, kind="ExternalInput")
    w = nc.dram_tensor("w", [128, 2 * 8 * 128], bf16, kind="ExternalInput")
    v16 = nc.dram_tensor("v16", [B, HPC, 128, TB * 65], bf16, kind="ExternalInput")
    v32 = nc.dram_tensor("v32", [B, HPC, 128, TB * 64], f32, kind="ExternalInput")
    tri_d = nc.dram_tensor("tri", [128, 128], bf16, kind="ExternalInput")
    bg_d = nc.dram_tensor("bg", [128, 2], f32, kind="ExternalInput")
    y = nc.dram_tensor("y", [B, T, HPC * 64], f32, kind="ExternalOutput")

    with tile.TileContext(nc) as tc:
        with (
            tc.tile_pool(name="consts", bufs=1) as consts,
            tc.tile_pool(name="xtp", bufs=9) as xtp,
            tc.tile_pool(name="qk", bufs=1) as qkp,
            tc.tile_pool(name="vp", bufs=1) as vp,
            tc.tile_pool(name="pt", bufs=46) as ptp,
            tc.tile_pool(name="pfx", bufs=4) as pfxp,
            tc.tile_pool(name="small", bufs=8) as small,
            tc.tile_pool(name="tmp", bufs=8) as tmp,
            tc.tile_pool(name="yst", bufs=1) as ystp,
            tc.tile_pool(name="proj_ps", bufs=1, space="PSUM") as proj_ps,
            tc.tile_pool(name="sc_ps", bufs=2, space="PSUM") as sc_ps,
            tc.tile_pool(name="uv_ps", bufs=2, space="PSUM") as uv_ps,
        ):
            tri_t = consts.tile([128, 128], bf16, tag="tri")
            nc.sync.dma_start(tri_t[:], tri_d[:])
            bg_t = consts.tile([128, 2], f32, tag="bg")
            nc.sync.dma_start(bg_t[:], bg_d[:])
            w_all = consts.tile([128, 2048], bf16, tag="w_all")
            nc.sync.dma_start(w_all[:], w[:])

            def w_t(m, c):
                return w_all[:, ds((m * 8 + c) * 128, 128)]

            v16_t = {}
            v32_t = {}
            for b in range(B):
                for hs in range(HPC):
                    v16_t[b, hs] = vp.tile([128, TB * 65], bf16,
                                           name=f"v16_{b}_{hs}", tag=f"v16_{b}_{hs}")
                    nc.sync.dma_start(v16_t[b, hs][:], v16[b, hs])
                    v32_t[b, hs] = vp.tile([128, TB * 64], f32,
                                           name=f"v32_{b}_{hs}", tag=f"v32_{b}_{hs}")
                    nc.sync.dma_start(v32_t[b, hs][:], v32[b, hs])

            qk_t = {}  # (b, m): m=0 -> Q2 [128, T], m=1 -> K2
            for b in range(B):
                for m in range(2):
                    qk_t[b, m] = qkp.tile([128, T], bf16, name=f"qk{b}{m}",
                                          tag=f"qk{b}{m}")

            yst = {}
            for b in range(B):
                for ib in range(TB):
                    yst[b, ib] = ystp.tile([128, 128], f32, name=f"yst{b}_{ib}",
                                           tag=f"yst{b}_{ib}")

            for b in range(B):
                xc = []
                for c in range(8):
                    t = xtp.tile([128, T], bf16)
                    nc.sync.dma_start(t[:], xT[b, c])
                    xc.append(t)
                # projection: qk[m][p, t] = sum_c w[m][c, p] * xT[c, t]
                for m in range(2):
                    for n in range(4):
                        ps = proj_ps.tile([128, 512], f32)
                        for c in range(8):
                            nc.tensor.matmul(
                                ps[:], w_t(m, c), xc[c][:, ts(n, 512)],
                                start=(c == 0), stop=(c == 7),
                            )
                        nc.scalar.copy(qk_t[b, m][:, ts(n, 512)], ps[:])

            for b in range(B):
                for hs in range(HPC):
                    p0 = 64 * hs
                    q2 = qk_t[b, 0]
                    k2 = qk_t[b, 1]
                    # scoresT wide blocks [j, 512 i] + exp -> PT tiles (bf16)
                    pt_t = {}
                    for jb in range(TB):
                        for iw in range(jb // 4, NW4):
                            sp = sc_ps.tile([128, 512], f32)
                            nc.tensor.matmul(
                                sp[:],
                                k2[ds(p0, 64), ts(jb, 128)],
                                q2[ds(p0, 64), ts(iw, 512)],
                                start=True, stop=True,
                            )
                            ptt = ptp.tile([128, 512], bf16)
                            nc.scalar.activation(ptt[:], sp[:], Exp)
                            if iw == jb // 4:
                                dcol = (jb % 4) * 128
                                nc.vector.tensor_mul(
                                    ptt[:, ds(dcol, 128)],
                                    ptt[:, ds(dcol, 128)],
                                    tri_t[:],
                                )
                            pt_t[jb, iw] = ptt

                    # block colsums s_jb[d] = sum_j v16[jb][j, d] -> [1, 65]
                    # each, via ones-column lhsT (tri col 127). Then prefix
                    # partial sums, all in partition 0.
                    css = []
                    for g in range(4):
                        cp = uv_ps.tile([1, 260], f32, tag="cs", bufs=1,
                                        name=f"cs{b}{hs}{g}")
                        for k in range(4):
                            jb = 4 * g + k
                            nc.tensor.matmul(
                                cp[0:1, ds(k * 65, 65)],
                                tri_t[:, ds(127, 1)],
                                v16_t[b, hs][:, ds(jb * 65, 65)],
                                start=True, stop=True,
                            )
                        cs_sb = pfxp.tile([1, 260], f32, tag="cs_sb",
                                          name=f"cssb{b}{hs}{g}")
                        nc.vector.tensor_copy(cs_sb[:], cp[:])
                        css.append(cs_sb)
                    pfx_sb = {}
                    prev = None
                    for ib in range(1, TB):
                        s = css[(ib - 1) // 4][0:1, ds(((ib - 1) % 4) * 65, 65)]
                        a = pfxp.tile([1, 65], f32, tag="acc", bufs=2,
                                      name=f"acc{b}{hs}{ib}")
                        if prev is None:
                            nc.vector.tensor_copy(a[:], s)
                        else:
                            nc.vector.tensor_add(a[:], prev[:], s)
                        prev = a
                        p16 = pfxp.tile([1, 65], bf16, tag=f"pfx{ib}",
                                        name=f"pfx{b}{hs}{ib}")
                        nc.vector.tensor_copy(p16[:], a[:])
                        pfx_sb[ib] = p16

                    for ib in range(TB):
                        vs = v16_t[b, hs][:, ds(ib * 65, 65)]
                        # U = P^T blocks @ v_ext (col 64 = softmax denom)
                        up = uv_ps.tile([128, 65], f32, tag="ups")
                        for jb in range(ib + 1):
                            ptt = pt_t[jb, ib // 4]
                            col = (ib % 4) * 128
                            nc.tensor.matmul(
                                up[:], ptt[:, ds(col, 128)],
                                v16_t[b, hs][:, ds(jb * 65, 65)],
                                start=(jb == 0), stop=(jb == ib),
                            )
                        # Lv = tri @ v_block + prefix (rank-1); col 64 = i+1
                        lp = uv_ps.tile([128, 65], f32, tag="lps")
                        nc.tensor.matmul(
                            lp[:], tri_t[:], vs,
                            start=True, stop=(ib == 0),
                        )
                        if ib > 0:
                            nc.tensor.matmul(
                                lp[:], tri_t[0:1, :], pfx_sb[ib][:],
                                start=False, stop=True,
                            )

                        r1 = small.tile([128, 1], f32, tag="r1")
                        nc.vector.reciprocal(r1[:], up[:, ds(64, 1)])
                        r2 = small.tile([128, 1], f32, tag="r2")
                        nc.vector.reciprocal(r2[:], lp[:, ds(64, 1)])
                        t1 = tmp.tile([128, 64], f32, tag="t1")
                        nc.vector.tensor_scalar(
                            t1[:], up[:, ds(0, 64)], r1[:], bg_t[:, ds(0, 1)],
                            mult, mult,
                        )
                        t2 = tmp.tile([128, 64], f32, tag="t2")
                        nc.vector.tensor_scalar(
                            t2[:], lp[:, ds(0, 64)], r2[:], bg_t[:, ds(1, 1)],
                            mult, mult,
                        )
                        t3 = tmp.tile([128, 64], f32, tag="t3")
                        nc.vector.tensor_sub(t3[:], t1[:], t2[:])
                        nc.vector.tensor_add(
                            yst[b, ib][:, ds(p0, 64)], t3[:],
                            v32_t[b, hs][:, ds(ib * 64, 64)],
                        )
                        if hs == HPC - 1:
                            nc.sync.dma_start(
                                y[b, ts(ib, 128), :], yst[b, ib][:]
                            )

    _split_excess_waits(nc)
    nc.finalize()
    return nc


def _prep_inputs(x, W_attn, alpha, beta, gamma):
    """Host-side sharding/layout prep. Returns per-core input maps."""
    bf = ml_dtypes.bfloat16
    x = np.asarray(x, dtype=np.float32)
    W_attn = np.asarray(W_attn, dtype=np.float32)
    alpha = float(alpha)
    beta = float(beta)
    gamma = float(gamma)

    # x^T per batch, c-chunked: [B, 8, 128, T] (shared by all cores)
    xT = np.ascontiguousarray(x.transpose(0, 2, 1).reshape(B, 8, 128, T)).astype(bf)

    tri = np.triu(np.ones((128, 128), dtype=np.float32)).astype(bf)  # j<=i
    bg = np.empty((128, 2), dtype=np.float32)
    bg[:, 0] = beta
    bg[:, 1] = gamma

    scale = HD ** -0.5
    in_maps = []
    for core in range(NCORES):
        h0 = HPC * core
        # w cols: [q(h0,h1) scaled | k(h0,h1)], packed [128c, (m,cchunk)*128]
        wq = W_attn[h0 * 64:(h0 + HPC) * 64, :].T * scale      # [C, 128]
        wk = W_attn[C + h0 * 64:C + (h0 + HPC) * 64, :].T      # [C, 128]
        wpack = np.stack([wq.reshape(8, 128, 128), wk.reshape(8, 128, 128)])
        # [2, 8, 128c, 128m] -> [128c, 2, 8, 128m]
        wpack = np.ascontiguousarray(wpack.transpose(2, 0, 1, 3).reshape(128, 2048))

        v = np.empty((B, HPC, TB, 128, 65), dtype=np.float32)
        v32 = np.empty((B, HPC, TB, 128, 64), dtype=np.float32)
        for b in range(B):
            for hs in range(HPC):
                h = h0 + hs
                vb = x[b][:, h * 64:(h + 1) * 64].reshape(TB, 128, 64)
                v[b, hs, :, :, :64] = vb
                v[b, hs, :, :, 64] = 1.0
                v32[b, hs] = alpha * vb
        # [B,HPC,TB,128,65] -> [B,HPC,128,TB*65]
        v = np.ascontiguousarray(v.transpose(0, 1, 3, 2, 4).reshape(B, HPC, 128, TB * 65))
        v32 = np.ascontiguousarray(v32.transpose(0, 1, 3, 2, 4).reshape(B, HPC, 128, TB * 64))
        in_maps.append({
            "xT": xT,
            "w": wpack.astype(bf),
            "v16": v.astype(bf),
            "v32": v32,
            "tri": tri,
            "bg": bg,
        })
    return in_maps


def kernel(x, W_attn, alpha, beta, gamma):
    global _PROGRAM, LAST_EXEC_NS, LAST_TRACE_DIR
    _install_patches()
    from concourse.bass_utils import run_bass_kernel_spmd

    if _PROGRAM is None:
        _PROGRAM = _build_program()
    nc = _PROGRAM

    in_maps = _prep_inputs(x, W_attn, alpha, beta, gamma)

    trace = os.environ.get("KERNEL_TRACE", "0") == "1"
    kwargs = {}
    if trace:
        trace_dir = os.environ.get("KERNEL_TRACE_DIR") or None
        if trace_dir:
            os.makedirs(trace_dir, exist_ok=True)
            kwargs["tmpdir"] = trace_dir
    res = run_bass_kernel_spmd(
        nc, in_maps, core_ids=list(range(NCORES)), trace=trace, **kwargs
    )
    LAST_EXEC_NS = res.exec_time_ns
    if trace and "tmpdir" in kwargs:
        LAST_TRACE_DIR = kwargs["tmpdir"]

    out = np.concatenate(
        [res.results[c]["y"] for c in range(NCORES)], axis=2
    )
    return np.ascontiguousarray(out, dtype=np.float32)


# revision 14
# speedup vs baseline: 1.6655x; 1.0743x over previous
"""CausalShapedAttention Trainium2 kernel.

y = beta * softmax(causal(q k^T / sqrt(hd))) @ v + alpha * v - gamma * MC @ v

where q,k = x @ W_attn^T (packed), v = x (reshaped to heads), MC = causal
uniform attention (row i: 1/(i+1) for j<=i).

Sharding: 16 heads / 8 cores = 2 heads per core, both batches per core.
Each core computes y columns [128c, 128c+128) of the [2, 2048, 1024] output.

Key identities used:
  softmax(s)_ij = exp(s_ij)/sum_j exp(s_ij)  (no max-sub needed: |s| < ~3)
  (MC @ v)_i = (sum_{j<=i} v_j) / (i+1)
  An extra ones-column appended to v makes the U matmul also produce the
  softmax denominator (col 64), and the Lv matmul produce i+1 (col 64).
  Lv (running causal sum of v) per 128-row block = tri @ v_block + prefix,
  where prefix is row 127 of the previous block's Lv (rank-1 matmul add).

All matmuls run with bf16 operands (fp32 matmul is 4 cycles/row on trn2 PE,
bf16 is 1) accumulating in fp32 PSUM; the dominant alpha*v output term is
added in fp32 from untouched input data. DRAM layouts are packed so every
DMA moves large contiguous lines (>=2KB per partition row).
"""

import os
import sys
import types

sys.path.insert(0, "/opt/trn_rl_repo")

import numpy as np
import ml_dtypes

B, T, C, H, HD = 2, 2048, 1024, 16, 64
NCORES = 8
HPC = H // NCORES            # heads per core = 2
TB = T // 128                # 16 row/col blocks
NW4 = T // 512               # 4 wide (512) column blocks

_PROGRAM = None
LAST_EXEC_NS = None
LAST_TRACE_DIR = None


def _install_patches():
    """Work around environment quirks:
    - walrus here rejects instructions with >1-2 sem waits (see
      _split_excess_waits).
    - antenv.axon_hooks is absent in this image: stub it and register the
      NTFF profile hook from trn_agent_boot so trace=True works.
    """
    try:
        import antenv  # noqa: F401
        if "antenv.axon_hooks" not in sys.modules:
            hooks_mod = types.ModuleType("antenv.axon_hooks")
            _h = [None]
            hooks_mod.set_axon_ntff_profile_hook = lambda h: _h.__setitem__(0, h)
            hooks_mod.get_axon_ntff_profile_hook = lambda: _h[0]
            sys.modules["antenv.axon_hooks"] = hooks_mod
            antenv.axon_hooks = hooks_mod
            from trn_agent_boot.trn_boot import _ntff_profile_via_ctypes
            hooks_mod.set_axon_ntff_profile_hook(
                _ntff_profile_via_ctypes("/opt/axon/libaxon_pjrt.so")
            )
        import concourse.bass_utils as bu
        bu.upload_artifacts = lambda d: d  # no artifact bucket here
    except Exception:
        pass


def _split_excess_waits(nc, limit=1):
    """walrus here rejects instructions with more than ~2 sem waits; split
    excess waits onto same-engine NoOps inserted just before the instruction
    (engine streams are per-engine program order, so semantics are identical).
    """
    import concourse.mybir as mybir

    n = 0
    for bb in nc.main_func.blocks:
        out = []
        for inst in bb.instructions:
            si = inst.sync_info
            if (
                si is not None
                and si.on_wait
                and len(si.on_wait) > limit
                and inst.engine != mybir.EngineType.Unassigned
            ):
                waits = list(si.on_wait)
                for w in waits[:-limit]:
                    n += 1
                    nop = mybir.InstNoOp(
                        name=f"{inst.name}-wsplit{n}",
                        engine=inst.engine,
                        ins=[], outs=[],
                        sync_info=mybir.SyncInfo(on_wait=[w], on_update=[]),
                    )
                    nc.register_instruction(nop)
                    out.append(nop)
                inst.sync_info = mybir.SyncInfo(
                    on_wait=waits[-limit:], on_update=list(si.on_update)
                )
            out.append(inst)
        bb.instructions = out


def _build_program():
    import concourse.bass as bass
    import concourse.mybir as mybir
    import concourse.tile as tile
    from concourse.bass import ts, ds

    f32 = mybir.dt.float32
    bf16 = mybir.dt.bfloat16
    Exp = mybir.ActivationFunctionType.Exp
    mult = mybir.AluOpType.mult

    nc = bass.Bass()
    # packed DRAM layouts: every row (partition line) is large + contiguous
    xT = nc.dram_tensor("xT", [B, 8, 128, T], bf16, kind="ExternalInput")
    w = nc.dram_tensor("w", [128, 2 * 8 * 128], bf16, kind="ExternalInput")
    v16 = nc.dram_tensor("v16", [B, HPC, 128, TB * 65], bf16, kind="ExternalInput")
    v32 = nc.dram_tensor("v32", [B, HPC, 128, TB * 64], f32, kind="ExternalInput")
    tri_d = nc.dram_tensor("tri", [128, 128], bf16, kind="ExternalInput")
    bg_d = nc.dram_tensor("bg", [128, 2], f32, kind="ExternalInput")
    y = nc.dram_tensor("y", [B, T, HPC * 64], f32, kind="ExternalOutput")

    with tile.TileContext(nc) as tc:
        with (
            tc.tile_pool(name="consts", bufs=1) as consts,
            tc.tile_pool(name="xtp", bufs=8) as xtp,
            tc.tile_pool(name="qk", bufs=1) as qkp,
            tc.tile_pool(name="vp", bufs=1) as vp,
            tc.tile_pool(name="pt", bufs=84) as ptp,
            tc.tile_pool(name="pfx", bufs=4) as pfxp,
            tc.tile_pool(name="small", bufs=8) as small,
            tc.tile_pool(name="tmp", bufs=8) as tmp,
            tc.tile_pool(name="yst", bufs=1) as ystp,
            tc.tile_pool(name="sc_ps", bufs=4, space="PSUM") as sc_ps,
            tc.tile_pool(name="uv_ps", bufs=2, space="PSUM") as uv_ps,
        ):
            tri_t = consts.tile([128, 128], bf16, tag="tri")
            nc.sync.dma_start(tri_t[:], tri_d[:])
            bg_t = consts.tile([128, 2], f32, tag="bg")
            nc.sync.dma_start(bg_t[:], bg_d[:])
            w_all = consts.tile([128, 2048], bf16, tag="w_all")
            nc.sync.dma_start(w_all[:], w[:])

            def w_t(m, c):
                return w_all[:, ds((m * 8 + c) * 128, 128)]

            v16_t = {}
            v32_t = {}
            for b in range(B):
                for hs in range(HPC):
                    v16_t[b, hs] = vp.tile([128, TB * 65], bf16,
                                           name=f"v16_{b}_{hs}", tag=f"v16_{b}_{hs}")
                    nc.sync.dma_start(v16_t[b, hs][:], v16[b, hs])
                    v32_t[b, hs] = vp.tile([128, TB * 64], f32,
                                           name=f"v32_{b}_{hs}", tag=f"v32_{b}_{hs}")
                    nc.sync.dma_start(v32_t[b, hs][:], v32[b, hs])

            qk_t = {}  # (b, m): m=0 -> Q2 [128, T], m=1 -> K2
            for b in range(B):
                for m in range(2):
                    qk_t[b, m] = qkp.tile([128, T], bf16, name=f"qk{b}{m}",
                                          tag=f"qk{b}{m}")

            yst = {}
            for b in range(B):
                for ib in range(TB):
                    yst[b, ib] = ystp.tile([128, 128], f32, name=f"yst{b}_{ib}",
                                           tag=f"yst{b}_{ib}")

            for b in range(B):
                xc = []
                for c in range(8):
                    t = xtp.tile([128, T], bf16)
                    nc.sync.dma_start(t[:], xT[b, c])
                    xc.append(t)
                # projection: qk[m][p, t] = sum_c w[m][c, p] * xT[c, t]
                for m in range(2):
                    for n in range(4):
                        ps = sc_ps.tile([128, 512], f32, name=f"ps{b}{m}{n}",
                                        tag="sp")
                        for c in range(8):
                            nc.tensor.matmul(
                                ps[:], w_t(m, c), xc[c][:, ts(n, 512)],
                                start=(c == 0), stop=(c == 7),
                            )
                        nc.scalar.copy(qk_t[b, m][:, ts(n, 512)], ps[:])

            pt_t = {}
            for b in range(B):
                q2 = qk_t[b, 0]
                k2 = qk_t[b, 1]
                # scoresT wide blocks [j, 512 i] + exp -> PT tiles (bf16).
                # Both heads interleaved: h0 weights sit in PE rows 0-63,
                # h1 in rows 64-127 (tile_position from base_partition), so
                # adjacent MMs overlap in the array. First block per jb is
                # trimmed to the causally valid columns.
                for jb in range(TB):
                    for iw in range(jb // 4, NW4):
                        dcol = (jb % 4) * 128 if iw == jb // 4 else 0
                        nw = 512 - dcol
                        for hs in range(HPC):
                            p0 = 64 * hs
                            sp = sc_ps.tile([128, 512], f32, tag="sp",
                                            name=f"sp{b}{hs}{jb}{iw}")
                            nc.tensor.matmul(
                                sp[:, ds(dcol, nw)],
                                k2[ds(p0, 64), ts(jb, 128)],
                                q2[ds(p0, 64), ds(iw * 512 + dcol, nw)],
                                start=True, stop=True,
                            )
                            ptt = ptp.tile([128, 512], bf16, tag="ptt",
                                           name=f"pt{b}{hs}{jb}{iw}")
                            nc.scalar.activation(
                                ptt[:, ds(dcol, nw)], sp[:, ds(dcol, nw)], Exp
                            )
                            if iw == jb // 4:
                                nc.vector.tensor_mul(
                                    ptt[:, ds(dcol, 128)],
                                    ptt[:, ds(dcol, 128)],
                                    tri_t[:],
                                )
                            pt_t[b, hs, jb, iw] = ptt

                for hs in range(HPC):
                    p0 = 64 * hs
                    # block colsums s_jb[d] = sum_j v16[jb][j, d] (4 blocks
                    # per matmul), via ones-column lhsT (tri col 127). Then
                    # prefix partial sums, all in partition 0.
                    css = []
                    for g in range(4):
                        cp = uv_ps.tile([1, 260], f32, tag="lps", bufs=2,
                                        name=f"cs{b}{hs}{g}")
                        nc.tensor.matmul(
                            cp[:], tri_t[:, ds(127, 1)],
                            v16_t[b, hs][:, ds(g * 260, 260)],
                            start=True, stop=True,
                        )
                        cs_sb = pfxp.tile([1, 260], f32, tag="cs_sb",
                                          name=f"cssb{b}{hs}{g}")
                        nc.vector.tensor_copy(cs_sb[:], cp[:])
                        css.append(cs_sb)
                    pfx_sb = {}
                    prev = None
                    for ib in range(1, TB):
                        s = css[(ib - 1) // 4][0:1, ds(((ib - 1) % 4) * 65, 65)]
                        a = pfxp.tile([1, 65], f32, tag="acc", bufs=2,
                                      name=f"acc{b}{hs}{ib}")
                        if prev is None:
                            nc.vector.tensor_copy(a[:], s)
                        else:
                            nc.vector.tensor_add(a[:], prev[:], s)
                        prev = a
                        p16 = pfxp.tile([1, 65], bf16, tag=f"pfx{ib}",
                                        name=f"pfx{b}{hs}{ib}")
                        nc.vector.tensor_copy(p16[:], a[:])
                        pfx_sb[ib] = p16

                    for ib in range(TB):
                        vs = v16_t[b, hs][:, ds(ib * 65, 65)]
                        # U = P^T blocks @ v_ext (col 64 = softmax denom)
                        up = uv_ps.tile([128, 65], f32, tag="ups")
                        for jb in range(ib + 1):
                            ptt = pt_t[b, hs, jb, ib // 4]
                            col = (ib % 4) * 128
                            nc.tensor.matmul(
                                up[:], ptt[:, ds(col, 128)],
                                v16_t[b, hs][:, ds(jb * 65, 65)],
                                start=(jb == 0), stop=(jb == ib),
                            )
                        # Lv = tri @ v_block + prefix (rank-1); col 64 = i+1
                        lp = uv_ps.tile([128, 65], f32, tag="lps")
                        nc.tensor.matmul(
                            lp[:], tri_t[:], vs,
                            start=True, stop=(ib == 0),
                        )
                        if ib > 0:
                            nc.tensor.matmul(
                                lp[:], tri_t[0:1, :], pfx_sb[ib][:],
                                start=False, stop=True,
                            )

                        r1 = small.tile([128, 1], f32, tag="r1")
                        nc.vector.reciprocal(r1[:], up[:, ds(64, 1)])
                        r2 = small.tile([128, 1], f32, tag="r2")
                        nc.vector.reciprocal(r2[:], lp[:, ds(64, 1)])
                        t1 = tmp.tile([128, 64], f32, tag="t1")
                        nc.vector.tensor_scalar(
                            t1[:], up[:, ds(0, 64)], r1[:], bg_t[:, ds(0, 1)],
                            mult, mult,
                        )
                        t2 = tmp.tile([128, 64], f32, tag="t2")
                        nc.vector.tensor_scalar(
                            t2[:], lp[:, ds(0, 64)], r2[:], bg_t[:, ds(1, 1)],
                            mult, mult,
                        )
                        t3 = tmp.tile([128, 64], f32, tag="t3")
                        nc.vector.tensor_sub(t3[:], t1[:], t2[:])
                        nc.vector.tensor_add(
                            yst[b, ib][:, ds(p0, 64)], t3[:],
                            v32_t[b, hs][:, ds(ib * 64, 64)],
                        )
                        if hs == HPC - 1:
                            nc.sync.dma_start(
                                y[b, ts(ib, 128), :], yst[b, ib][:]
                            )

    _split_excess_waits(nc)
    nc.finalize()
    return nc


def _prep_inputs(x, W_attn, alpha, beta, gamma):
    """Host-side sharding/layout prep. Returns per-core input maps."""
    bf = ml_dtypes.bfloat16
    x = np.asarray(x, dtype=np.float32)
    W_attn = np.asarray(W_attn, dtype=np.float32)
    alpha = float(alpha)
    beta = float(beta)
    gamma = float(gamma)

    # x^T per batch, c-chunked: [B, 8, 128, T] (shared by all cores)
    xT = np.ascontiguousarray(x.transpose(0, 2, 1).reshape(B, 8, 128, T)).astype(bf)

    tri = np.triu(np.ones((128, 128), dtype=np.float32)).astype(bf)  # j<=i
    bg = np.empty((128, 2), dtype=np.float32)
    bg[:, 0] = beta
    bg[:, 1] = gamma

    scale = HD ** -0.5
    in_maps = []
    for core in range(NCORES):
        h0 = HPC * core
        # w cols: [q(h0,h1) scaled | k(h0,h1)], packed [128c, (m,cchunk)*128]
        wq = W_attn[h0 * 64:(h0 + HPC) * 64, :].T * scale      # [C, 128]
        wk = W_attn[C + h0 * 64:C + (h0 + HPC) * 64, :].T      # [C, 128]
        wpack = np.stack([wq.reshape(8, 128, 128), wk.reshape(8, 128, 128)])
        # [2, 8, 128c, 128m] -> [128c, 2, 8, 128m]
        wpack = np.ascontiguousarray(wpack.transpose(2, 0, 1, 3).reshape(128, 2048))

        v = np.empty((B, HPC, TB, 128, 65), dtype=np.float32)
        v32 = np.empty((B, HPC, TB, 128, 64), dtype=np.float32)
        for b in range(B):
            for hs in range(HPC):
                h = h0 + hs
                vb = x[b][:, h * 64:(h + 1) * 64].reshape(TB, 128, 64)
                v[b, hs, :, :, :64] = vb
                v[b, hs, :, :, 64] = 1.0
                v32[b, hs] = alpha * vb
        # [B,HPC,TB,128,65] -> [B,HPC,128,TB*65]
        v = np.ascontiguousarray(v.transpose(0, 1, 3, 2, 4).reshape(B, HPC, 128, TB * 65))
        v32 = np.ascontiguousarray(v32.transpose(0, 1, 3, 2, 4).reshape(B, HPC, 128, TB * 64))
        in_maps.append({
            "xT": xT,
            "w": wpack.astype(bf),
            "v16": v.astype(bf),
            "v32": v32,
            "tri": tri,
            "bg": bg,
        })
    return in_maps


def kernel(x, W_attn, alpha, beta, gamma):
    global _PROGRAM, LAST_EXEC_NS, LAST_TRACE_DIR
    _install_patches()
    from concourse.bass_utils import run_bass_kernel_spmd

    if _PROGRAM is None:
        _PROGRAM = _build_program()
    nc = _PROGRAM

    in_maps = _prep_inputs(x, W_attn, alpha, beta, gamma)

    trace = os.environ.get("KERNEL_TRACE", "0") == "1"
    kwargs = {}
    if trace:
        trace_dir = os.environ.get("KERNEL_TRACE_DIR") or None
        if trace_dir:
            os.makedirs(trace_dir, exist_ok=True)
            kwargs["tmpdir"] = trace_dir
    res = run_bass_kernel_spmd(
        nc, in_maps, core_ids=list(range(NCORES)), trace=trace, **kwargs
    )
    LAST_EXEC_NS = res.exec_time_ns
    if trace and "tmpdir" in kwargs:
        LAST_TRACE_DIR = kwargs["tmpdir"]

    out = np.concatenate(
        [res.results[c]["y"] for c in range(NCORES)], axis=2
    )
    return np.ascontiguousarray(out, dtype=np.float32)
